# revision 16
# baseline (speedup 1.0000x reference)
"""Trainium2 Bass kernel for ViT-style multi-head attention with relative
position bias.

Problem (per full input):
  x        [8, 1024, 768] f32
  W_qkv    [768, 2304]    f32
  W_proj   [768, 768]     f32
  b_proj   [768]          f32
  bias_table [2047, 12]   f32
  rel_index  [1024, 1024] int32

Sharding: pure data parallel - one batch element per NeuronCore (B=8 over 8
cores), weights replicated. No collectives.

v2 design (vs v1 baseline at ~314us):
  - The rel-pos multiplier E[h, j, i] = exp(t_h[i - j + 1023]) is Toeplitz,
    so instead of streaming the 25MB expanded table from HBM, the host
    builds a SKEWED table S[h, p, u] = exp(t_h[u - p + 127]) of width 1920.
    Then E for score block (jc) is the plain slice S[h, :, off:off+1024]
    with off = 896 - 128*jc -- a uniform AP, zero expansion cost. Streamed
    per head-pair (983KB) instead of 25MB total.
  - PV is FLIPPED: stationary = pT chunk [j, i-block], moving = v [j, d+1]
    (with a ones column), giving out[i, d] per 128-i-block with the softmax
    denominator landing in a PSUM COLUMN -> per-partition reciprocal
    ([128,2] DVE op) + stride-0-broadcast tensor_mul normalize. This kills
    the v1 DRAM-bounce reciprocal dance (96 small DMAs).
  - The normalized o[i, c] tiles are PE-transposed ([128,128] via identity)
    into outT[c, i] for the projection, pipelined one slot behind PV so the
    PE never waits on the DVE chain.
  - exp runs as one ACTIVATE per (pair, jc) at FD=2048 from a 4-bank PSUM
    tile (amortizes the 352-cycle ACT overhead).
  - Tail: PV(last pair, ib) -> transpose -> proj(nj=ib) pipelined.

Emission is software-pipelined at head-pair granularity: scores(hp) with
filler slots running PV(hp-1), qk(hp+1), transposes. Matmul groups must NOT
be emitted between the row-tiled K=64 score matmuls of a pair (HW hazard);
whole groups at jc boundaries are safe.
"""

import numpy as np
import ml_dtypes

B = 8
N = 1024
C = 768
H = 12
DH = 64
P = 128
KC = C // P          # 6 contraction chunks of 128 over C
NJ = N // P          # 8 chunks of 128 over the j (key) axis
NT = N // 512        # 2 tiles of 512 over the i (query) axis
HP = H // 2          # 6 head pairs
T5 = 512
SW = 1920            # skewed-table width

_BUILT = {}


def _build_nc():
    from contextlib import ExitStack
    import concourse.bass as bass
    import concourse.mybir as mybir
    import concourse.tile as tile
    from concourse import bacc
    from concourse import masks

    bf16 = mybir.dt.bfloat16
    f32 = mybir.dt.float32
    Exp = mybir.ActivationFunctionType.Exp

    nc = bacc.Bacc("TRN2", target_bir_lowering=False, debug=False)

    xT_d = nc.dram_tensor("xT", [C, N], bf16, kind="ExternalInput")
    w_d = nc.dram_tensor("wqkv", [C, 3 * C], bf16, kind="ExternalInput")
    wp_d = nc.dram_tensor("wproj", [C, C], bf16, kind="ExternalInput")
    bp_d = nc.dram_tensor("bproj", [C], f32, kind="ExternalInput")
    sk_d = nc.dram_tensor("skew", [HP, 2, P, SW], bf16, kind="ExternalInput")
    out_d = nc.dram_tensor("out", [N, C], f32, kind="ExternalOutput")

    with ExitStack() as ctx:
        tc = ctx.enter_context(tile.TileContext(nc))

        singles = ctx.enter_context(tc.tile_pool(name="singles", bufs=1))
        pt_pool = ctx.enter_context(tc.tile_pool(name="pt_pool", bufs=2))
        s_pool = ctx.enter_context(tc.tile_pool(name="s_pool", bufs=2))
        es_pool = ctx.enter_context(tc.tile_pool(name="es_pool", bufs=3))
        on_pool = ctx.enter_context(tc.tile_pool(name="on_pool", bufs=3))
        rec_pool = ctx.enter_context(tc.tile_pool(name="rec_pool", bufs=3))
        ost_pool = ctx.enter_context(tc.tile_pool(name="ost_pool", bufs=2))
        sc_ps = ctx.enter_context(tc.tile_pool(name="sc_ps", bufs=2, space="PSUM"))
        mm_ps = ctx.enter_context(tc.tile_pool(name="mm_ps", bufs=2, space="PSUM"))
        pv_ps = ctx.enter_context(tc.tile_pool(name="pv_ps", bufs=2, space="PSUM"))

        # ---- resident SBUF tensors (loads chunked so the PE starts early) --
        xT_sb = singles.tile([P, KC, N], bf16)
        xT_r = xT_d.ap().rearrange("(kc p) n -> p kc n", p=P)
        w_sb = singles.tile([P, KC, 3 * C], bf16)
        w_r = w_d.ap().rearrange("(kc p) d -> p kc d", p=P)
        # whole-tensor input DMAs split across the two HWDGE rings (sync +
        # scalar queues) -- per-chunk DMAs cost ~650ns of issue time each,
        # which serialized the lead in earlier versions
        nc.sync.dma_start(out=xT_sb, in_=xT_r)
        wp_sb = singles.tile([P, KC, C], bf16)
        bp_sb = singles.tile([P, C], f32)

        ident = singles.tile([P, P], bf16)
        masks.make_identity(nc, ident[:, :])

        qT_sb = singles.tile([P, HP, N], bf16)   # chunk hp = heads (2hp, 2hp+1)
        kT_sb = singles.tile([P, HP, N], bf16)
        v_sb = singles.tile([P, NJ, H, DH + 1], bf16)  # col DH = ones
        nc.vector.memset(v_sb[:, :, :, DH:DH + 1], 1.0)
        oT_sb = singles.tile([P, KC, N], bf16)   # kc chunk == pair hp

        # skew-table tiles, prefetched one pair ahead; pair 0 rides the
        # scalar HWDGE ring so it lands before the first TT multiply
        s_tiles = [None] * HP
        def prefetch_skew(hp, eng=None):
            s_tiles[hp] = s_pool.tile([P, 2, SW], bf16, tag="sk",
                                      name=f"sk_{hp}")
            (eng or nc.sync).dma_start(
                out=s_tiles[hp],
                in_=sk_d.ap()[hp].rearrange("t p u -> p t u"))

        # ---- matmul group emitters (closures; emitted in interleaved order) --

        def v_group(nj, et):
            e0 = et * 384
            def emit():
                ps = mm_ps.tile([P, 384], f32, tag="mm", name=f"ps_v_{nj}_{et}")
                for kc in range(KC):
                    nc.tensor.matmul(
                        ps,
                        xT_sb[:, kc, nj * P:(nj + 1) * P],
                        w_sb[:, kc, 2 * C + e0:2 * C + e0 + 384],
                        start=(kc == 0), stop=(kc == KC - 1),
                    )
                h0 = e0 // DH
                nc.vector.tensor_copy(
                    out=v_sb[:, nj, h0:h0 + 6, 0:DH],
                    in_=ps.rearrange("p (h d) -> p h d", h=6),
                )
            return emit

        def qk_group(hp, which, it):
            col0 = hp * P if which == 0 else C + hp * P
            def emit():
                d = qT_sb if which == 0 else kT_sb
                ps = mm_ps.tile([P, T5], f32, tag="mm",
                                name=f"ps_qk_{hp}_{which}_{it}")
                for kc in range(KC):
                    nc.tensor.matmul(
                        ps,
                        w_sb[:, kc, col0:col0 + P],
                        xT_sb[:, kc, it * T5:(it + 1) * T5],
                        start=(kc == 0), stop=(kc == KC - 1),
                    )
                nc.vector.tensor_copy(
                    out=d[:, hp, it * T5:(it + 1) * T5], in_=ps)
            return emit

        # pending (hp, ib, o_n) tiles awaiting PE transpose into oT_sb
        pend = []

        def pv_group(hp, ib):
            """Flipped PV for both heads of pair hp, query block ib."""
            def emit():
                pt = pts[hp]
                pv = pv_ps.tile([P, 2, DH + 1], f32, tag="pv",
                                name=f"pv_{hp}_{ib}")
                for t in range(2):
                    h = 2 * hp + t
                    for jc in range(NJ):
                        nc.tensor.matmul(
                            pv[:, t, :],
                            pt[:, jc, t, ib * P:(ib + 1) * P],
                            v_sb[:, jc, h, :],
                            start=(jc == 0), stop=(jc == NJ - 1),
                        )
                rec = rec_pool.tile([P, 2, 1], f32, tag="rec",
                                    name=f"rec_{hp}_{ib}")
                nc.vector.reciprocal(rec, pv[:, :, DH:DH + 1])
                o_n = on_pool.tile([P, 2, DH], bf16, tag="on",
                                   name=f"on_{hp}_{ib}")
                nc.vector.tensor_mul(
                    out=o_n,
                    in0=pv[:, :, 0:DH],
                    in1=rec[:, :, :].broadcast_to([P, 2, DH]),
                )
                pend.append((hp, ib, o_n))
            return emit

        def flush_tp(lag=1):
            """Transpose+evict the oldest pending o_n tile (keeping `lag`
            entries so the PE never waits on the DVE normalize chain)."""
            while len(pend) > lag:
                hp, ib, o_n = pend.pop(0)
                tp = pv_ps.tile([P, P], bf16, tag="pv", name=f"tp_{hp}_{ib}")
                nc.tensor.transpose(
                    tp, o_n[:, :, :].rearrange("p t d -> p (t d)"), ident)
                nc.vector.tensor_copy(
                    out=oT_sb[:, hp, ib * P:(ib + 1) * P], in_=tp)

        def proj_group(nj):
            # proj PSUM comes from sc_ps (dead after the last scores phase)
            # so the et groups double-buffer instead of waiting on bias-adds
            def emit():
                osb = ost_pool.tile([P, C], f32, tag="osb",
                                    name=f"osb_{nj}")
                for et in range(2):
                    pp = sc_ps.tile([P, 2, T5], f32, tag="sc",
                                    name=f"pp_{nj}_{et}")[:, 0, 0:384]
                    for kc in range(KC):
                        nc.tensor.matmul(
                            pp,
                            oT_sb[:, kc, nj * P:(nj + 1) * P],
                            wp_sb[:, kc, et * 384:(et + 1) * 384],
                            start=(kc == 0), stop=(kc == KC - 1),
                        )
                    nc.vector.tensor_add(
                        out=osb[:, et * 384:(et + 1) * 384],
                        in0=pp,
                        in1=bp_sb[:, et * 384:(et + 1) * 384],
                    )
                nc.sync.dma_start(
                    out=out_d.ap()[nj * P:(nj + 1) * P, :], in_=osb)
            return emit

        # ---- emission: software-pipelined at pair granularity ----
        # Per steady-state pair hp: scores(hp) jc-groups with slots running
        # PV(hp-1), qk(hp+1), and deferred transposes. NOTE: nothing may be
        # emitted BETWEEN the row-tiled K=64 score matmuls of a jc group.

        def scores_phase(hp, pt, slots):
            s_t = s_tiles[hp]
            for jc in range(NJ):
                off = 896 - 128 * jc
                for it in range(NT):
                    # [P, 2, 512] = 2 PSUM banks, double-buffered across
                    # (jc, it) so scores MMs overlap the previous ACTIVATE
                    sc = sc_ps.tile([P, 2, T5], f32, tag="sc",
                                    name=f"sc_{hp}_{jc}_{it}")
                    nc.tensor.matmul(
                        sc[:, 0, :],
                        kT_sb[0:DH, hp, jc * P:(jc + 1) * P],
                        qT_sb[0:DH, hp, it * T5:(it + 1) * T5],
                        start=True, stop=True,
                    )
                    nc.tensor.matmul(
                        sc[:, 1, :],
                        kT_sb[DH:P, hp, jc * P:(jc + 1) * P],
                        qT_sb[DH:P, hp, it * T5:(it + 1) * T5],
                        start=True, stop=True,
                    )
                    es = es_pool.tile([P, 2, T5], bf16, tag="es",
                                      name=f"es_{hp}_{jc}_{it}")
                    nc.scalar.activation(out=es, in_=sc, func=Exp)
                    nc.vector.tensor_mul(
                        out=pt[:, jc, :, it * T5:(it + 1) * T5],
                        in0=es,
                        in1=s_t[:, :, off + it * T5:off + (it + 1) * T5])
                for fn in slots[jc]:
                    fn()

        prefetch_skew(0, eng=nc.scalar)
        # whole-W load: contiguous destination -> 128 large descriptors
        # (a column-sliced dst costs ~16ns/descriptor of HWDGE issue time)
        nc.scalar.dma_start(out=w_sb, in_=w_r)
        nc.scalar.dma_start(out=wp_sb, in_=wp_d.ap().rearrange("(kc p) d -> p kc d", p=P))
        bp_ap = bp_d.ap()
        bp_bcast = bass.AP(tensor=bp_ap.tensor, offset=bp_ap.offset,
                           ap=[[0, P], *bp_ap.ap])
        nc.gpsimd.dma_start(out=bp_sb, in_=bp_bcast)
        # dummy exp: pulls the ACT table load off the critical path
        dummy = rec_pool.tile([1, 1], f32, tag="rec", name="act_warm")
        nc.scalar.activation(out=dummy, in_=ident[0:1, 0:1], func=Exp)

        for which in range(2):
            for it in range(NT):
                qk_group(0, which, it)()

        pts = [None] * HP
        vg = [v_group(nj, et) for nj in range(NJ) for et in range(2)]
        for g in vg[:8]:
            g()
        vrest = vg[8:]

        fl = lambda: flush_tp(lag=1)
        for hp in range(HP):
            if hp + 1 < HP:
                prefetch_skew(hp + 1)
            slots = [[] for _ in range(NJ)]
            if hp + 1 < HP:
                qks = [qk_group(hp + 1, w, it)
                       for w in range(2) for it in range(NT)]
                slots[1].append(qks[0])
                slots[3].append(qks[1])
                slots[5].append(qks[2])
                slots[7].append(qks[3])
            if hp > 0:
                for k in range(NJ):
                    slots[k].insert(0, pv_group(hp - 1, k))
                    slots[k].append(fl)
            else:
                for k in range(NJ):
                    slots[k].insert(0, vrest[k])
            pts[hp] = pt_pool.tile([P, NJ, 2, N], bf16, tag="pt",
                                   name=f"pt_{hp}")
            scores_phase(hp, pts[hp], slots)

        # tail: PV(last pair) -> transpose -> proj, pipelined per i-block
        def flush_until(hp, ib):
            while pend:
                h0, i0, o_n = pend.pop(0)
                tp = pv_ps.tile([P, P], bf16, tag="pv", name=f"tp_{h0}_{i0}")
                nc.tensor.transpose(
                    tp, o_n[:, :, :].rearrange("p t d -> p (t d)"), ident)
                nc.vector.tensor_copy(
                    out=oT_sb[:, h0, i0 * P:(i0 + 1) * P], in_=tp)
                if h0 == hp and i0 == ib:
                    break

        for ib in range(NJ):
            pv_group(HP - 1, ib)()
            if ib >= 1:
                flush_until(HP - 1, ib - 1)
                proj_group(ib - 1)()
        flush_until(HP - 1, NJ - 1)
        proj_group(NJ - 1)()

    nc.finalize()
    return nc


def _get_nc():
    if "nc" not in _BUILT:
        _BUILT["nc"] = _build_nc()
    return _BUILT["nc"]


def _prep_inputs(x, W_qkv, W_proj, b_proj, bias_table, rel_index):
    bf = ml_dtypes.bfloat16
    x = np.asarray(x, dtype=np.float32)
    W_qkv = np.asarray(W_qkv, dtype=np.float32)
    W_proj = np.asarray(W_proj, dtype=np.float32)
    b_proj = np.asarray(b_proj, dtype=np.float32)
    bias_table = np.asarray(bias_table, dtype=np.float32)

    xT = np.ascontiguousarray(x.transpose(0, 2, 1)).astype(bf)       # [B, C, N]
    wq = W_qkv.copy()
    wq[:, :C] *= DH ** -0.5          # fold the attention scale into W_q
    wq = wq.astype(bf)
    wp = W_proj.astype(bf)

    # skewed multiplier table: S[h, p, u] = exp(t_h[p - u + 1919]);
    # E[h, j=128*jc+p, i] == S[h, p, i + 896 - 128*jc]
    t_exp = np.exp(bias_table)                      # [2047, H]
    idx = np.arange(P)[:, None] - np.arange(SW)[None, :] + 1919  # [P, SW]
    sk = t_exp[idx, :]                              # [P, SW, H]
    sk = np.ascontiguousarray(sk.transpose(2, 0, 1))  # [H, P, SW]
    sk = sk.reshape(HP, 2, P, SW).astype(bf)

    shared = {"wqkv": wq, "wproj": wp, "bproj": b_proj, "skew": sk}
    in_maps = []
    for b in range(B):
        m = dict(shared)
        m["xT"] = np.ascontiguousarray(xT[b])
        in_maps.append(m)
    return in_maps


def run(x, W_qkv, W_proj, b_proj, bias_table, rel_index, trace=False):
    """Returns (output [B, N, C] f32, exec_time_ns or None)."""
    from concourse.bass_utils import run_bass_kernel_spmd

    nc = _get_nc()
    in_maps = _prep_inputs(x, W_qkv, W_proj, b_proj, bias_table, rel_index)
    res = run_bass_kernel_spmd(nc, in_maps, core_ids=list(range(B)), trace=trace)
    out = np.stack([r["out"] for r in res.results]).astype(np.float32)
    return out, res.exec_time_ns


def kernel(x, W_qkv, W_proj, b_proj, bias_table, rel_index):
    out, _ = run(x, W_qkv, W_proj, b_proj, bias_table, rel_index, trace=False)
    return out


# revision 17
# speedup vs baseline: 1.1796x; 1.1796x over previous
"""Trainium2 Bass kernel for ViT-style multi-head attention with relative
position bias.

Problem (per full input):
  x        [8, 1024, 768] f32
  W_qkv    [768, 2304]    f32
  W_proj   [768, 768]     f32
  b_proj   [768]          f32
  bias_table [2047, 12]   f32
  rel_index  [1024, 1024] int32

Sharding: pure data parallel - one batch element per NeuronCore (B=8 over 8
cores), weights replicated. No collectives.

v2 design (vs v1 baseline at ~314us):
  - The rel-pos multiplier E[h, j, i] = exp(t_h[i - j + 1023]) is Toeplitz,
    so instead of streaming the 25MB expanded table from HBM, the host
    builds a SKEWED table S[h, p, u] = exp(t_h[u - p + 127]) of width 1920.
    Then E for score block (jc) is the plain slice S[h, :, off:off+1024]
    with off = 896 - 128*jc -- a uniform AP, zero expansion cost. Streamed
    per head-pair (983KB) instead of 25MB total.
  - PV is FLIPPED: stationary = pT chunk [j, i-block], moving = v [j, d+1]
    (with a ones column), giving out[i, d] per 128-i-block with the softmax
    denominator landing in a PSUM COLUMN -> per-partition reciprocal
    ([128,2] DVE op) + stride-0-broadcast tensor_mul normalize. This kills
    the v1 DRAM-bounce reciprocal dance (96 small DMAs).
  - The normalized o[i, c] tiles are PE-transposed ([128,128] via identity)
    into outT[c, i] for the projection, pipelined one slot behind PV so the
    PE never waits on the DVE chain.
  - exp runs as one ACTIVATE per (pair, jc) at FD=2048 from a 4-bank PSUM
    tile (amortizes the 352-cycle ACT overhead).
  - Tail: PV(last pair, ib) -> transpose -> proj(nj=ib) pipelined.

Emission is software-pipelined at head-pair granularity: scores(hp) with
filler slots running PV(hp-1), qk(hp+1), transposes. Matmul groups must NOT
be emitted between the row-tiled K=64 score matmuls of a pair (HW hazard);
whole groups at jc boundaries are safe.
"""

import numpy as np
import ml_dtypes

B = 8
N = 1024
C = 768
H = 12
DH = 64
P = 128
KC = C // P          # 6 contraction chunks of 128 over C
NJ = N // P          # 8 chunks of 128 over the j (key) axis
NT = N // 512        # 2 tiles of 512 over the i (query) axis
HP = H // 2          # 6 head pairs
T5 = 512
SW = 1920            # skewed-table width

_BUILT = {}


def _build_nc():
    from contextlib import ExitStack
    import concourse.bass as bass
    import concourse.mybir as mybir
    import concourse.tile as tile
    from concourse import bacc
    from concourse import masks

    bf16 = mybir.dt.bfloat16
    f32 = mybir.dt.float32
    Exp = mybir.ActivationFunctionType.Exp

    nc = bacc.Bacc("TRN2", target_bir_lowering=False, debug=False)

    xT_d = nc.dram_tensor("xT", [C, N], bf16, kind="ExternalInput")
    w_d = nc.dram_tensor("wqkv", [C, 3 * C], bf16, kind="ExternalInput")
    wp_d = nc.dram_tensor("wproj", [C, C], bf16, kind="ExternalInput")
    bp_d = nc.dram_tensor("bproj", [C], f32, kind="ExternalInput")
    sk_d = nc.dram_tensor("skew", [HP, 2, P, SW], bf16, kind="ExternalInput")
    out_d = nc.dram_tensor("out", [N, C], f32, kind="ExternalOutput")

    with ExitStack() as ctx:
        tc = ctx.enter_context(tile.TileContext(nc))

        singles = ctx.enter_context(tc.tile_pool(name="singles", bufs=1))
        pt_pool = ctx.enter_context(tc.tile_pool(name="pt_pool", bufs=2))
        s_pool = ctx.enter_context(tc.tile_pool(name="s_pool", bufs=2))
        es_pool = ctx.enter_context(tc.tile_pool(name="es_pool", bufs=3))
        on_pool = ctx.enter_context(tc.tile_pool(name="on_pool", bufs=3))
        rec_pool = ctx.enter_context(tc.tile_pool(name="rec_pool", bufs=3))
        ost_pool = ctx.enter_context(tc.tile_pool(name="ost_pool", bufs=2))
        sc_ps = ctx.enter_context(tc.tile_pool(name="sc_ps", bufs=2, space="PSUM"))
        mm_ps = ctx.enter_context(tc.tile_pool(name="mm_ps", bufs=2, space="PSUM"))
        pv_ps = ctx.enter_context(tc.tile_pool(name="pv_ps", bufs=2, space="PSUM"))

        # ---- resident SBUF tensors (loads chunked so the PE starts early) --
        xT_sb = singles.tile([P, KC, N], bf16)
        xT_r = xT_d.ap().rearrange("(kc p) n -> p kc n", p=P)
        w_sb = singles.tile([P, KC, 3 * C], bf16)
        w_r = w_d.ap().rearrange("(kc p) d -> p kc d", p=P)
        # whole-tensor input DMAs split across the two HWDGE rings (sync +
        # scalar queues) -- per-chunk DMAs cost ~650ns of issue time each,
        # which serialized the lead in earlier versions
        nc.sync.dma_start(out=xT_sb, in_=xT_r)
        wp_sb = singles.tile([P, KC, C], bf16)
        bp_sb = singles.tile([P, C], f32)

        ident = singles.tile([P, P], bf16)
        masks.make_identity(nc, ident[:, :])

        qT_sb = singles.tile([P, HP, N], bf16)   # chunk hp = heads (2hp, 2hp+1)
        kT_sb = singles.tile([P, HP, N], bf16)
        v_sb = singles.tile([P, NJ, H, DH + 1], bf16)  # col DH = ones
        nc.vector.memset(v_sb[:, :, :, DH:DH + 1], 1.0)
        oT_sb = singles.tile([P, KC, N], bf16)   # kc chunk == pair hp

        # skew-table tiles, prefetched one pair ahead; pair 0 rides the
        # scalar HWDGE ring so it lands before the first TT multiply
        s_tiles = [None] * HP
        def prefetch_skew(hp, eng=None):
            s_tiles[hp] = s_pool.tile([P, 2, SW], bf16, tag="sk",
                                      name=f"sk_{hp}")
            (eng or nc.sync).dma_start(
                out=s_tiles[hp],
                in_=sk_d.ap()[hp].rearrange("t p u -> p t u"))

        # ---- matmul group emitters (closures; emitted in interleaved order) --

        def v_group(nj, et):
            e0 = et * 384
            def emit():
                ps = mm_ps.tile([P, 384], f32, tag="mm", name=f"ps_v_{nj}_{et}")
                for kc in range(KC):
                    nc.tensor.matmul(
                        ps,
                        xT_sb[:, kc, nj * P:(nj + 1) * P],
                        w_sb[:, kc, 2 * C + e0:2 * C + e0 + 384],
                        start=(kc == 0), stop=(kc == KC - 1),
                    )
                h0 = e0 // DH
                nc.vector.tensor_copy(
                    out=v_sb[:, nj, h0:h0 + 6, 0:DH],
                    in_=ps.rearrange("p (h d) -> p h d", h=6),
                )
            return emit

        def qk_group(hp, which, it):
            col0 = hp * P if which == 0 else C + hp * P
            def emit():
                d = qT_sb if which == 0 else kT_sb
                ps = mm_ps.tile([P, T5], f32, tag="mm",
                                name=f"ps_qk_{hp}_{which}_{it}")
                for kc in range(KC):
                    nc.tensor.matmul(
                        ps,
                        w_sb[:, kc, col0:col0 + P],
                        xT_sb[:, kc, it * T5:(it + 1) * T5],
                        start=(kc == 0), stop=(kc == KC - 1),
                    )
                nc.vector.tensor_copy(
                    out=d[:, hp, it * T5:(it + 1) * T5], in_=ps)
            return emit

        # pending (hp, ib, o_n) tiles awaiting PE transpose into oT_sb
        pend = []

        def pv_group(hp, ib):
            """Flipped PV for both heads of pair hp, query block ib."""
            def emit():
                pt = pts[hp]
                pv = pv_ps.tile([P, 2, DH + 1], f32, tag="pv",
                                name=f"pv_{hp}_{ib}")
                for t in range(2):
                    h = 2 * hp + t
                    for jc in range(NJ):
                        nc.tensor.matmul(
                            pv[:, t, :],
                            pt[:, jc, t, ib * P:(ib + 1) * P],
                            v_sb[:, jc, h, :],
                            start=(jc == 0), stop=(jc == NJ - 1),
                        )
                rec = rec_pool.tile([P, 2, 1], f32, tag="rec",
                                    name=f"rec_{hp}_{ib}")
                nc.vector.reciprocal(rec, pv[:, :, DH:DH + 1])
                o_n = on_pool.tile([P, 2, DH], bf16, tag="on",
                                   name=f"on_{hp}_{ib}")
                nc.vector.tensor_mul(
                    out=o_n,
                    in0=pv[:, :, 0:DH],
                    in1=rec[:, :, :].broadcast_to([P, 2, DH]),
                )
                pend.append((hp, ib, o_n))
            return emit

        def flush_tp(lag=1):
            """Transpose+evict the oldest pending o_n tile (keeping `lag`
            entries so the PE never waits on the DVE normalize chain)."""
            while len(pend) > lag:
                hp, ib, o_n = pend.pop(0)
                tp = mm_ps.tile([P, P], bf16, tag="mm", name=f"tp_{hp}_{ib}")
                nc.tensor.transpose(
                    tp, o_n[:, :, :].rearrange("p t d -> p (t d)"), ident)
                nc.vector.tensor_copy(
                    out=oT_sb[:, hp, ib * P:(ib + 1) * P], in_=tp)

        def proj_group(nj):
            # proj PSUM comes from sc_ps (dead after the last scores phase)
            # so the et groups double-buffer instead of waiting on bias-adds
            def emit():
                osb = ost_pool.tile([P, C], f32, tag="osb",
                                    name=f"osb_{nj}")
                for et in range(2):
                    pp = sc_ps.tile([P, 2, T5], f32, tag="sc",
                                    name=f"pp_{nj}_{et}")[:, 0, 0:384]
                    for kc in range(KC):
                        nc.tensor.matmul(
                            pp,
                            oT_sb[:, kc, nj * P:(nj + 1) * P],
                            wp_sb[:, kc, et * 384:(et + 1) * 384],
                            start=(kc == 0), stop=(kc == KC - 1),
                        )
                    nc.vector.tensor_add(
                        out=osb[:, et * 384:(et + 1) * 384],
                        in0=pp,
                        in1=bp_sb[:, et * 384:(et + 1) * 384],
                    )
                nc.sync.dma_start(
                    out=out_d.ap()[nj * P:(nj + 1) * P, :], in_=osb)
            return emit

        # ---- emission: software-pipelined at pair granularity ----
        # Per steady-state pair hp: scores(hp) jc-groups with slots running
        # PV(hp-1), qk(hp+1), and deferred transposes. NOTE: nothing may be
        # emitted BETWEEN the row-tiled K=64 score matmuls of a jc group.

        def scores_phase(hp, pt, slots):
            s_t = s_tiles[hp]
            for jc in range(NJ):
                off = 896 - 128 * jc
                for it in range(NT):
                    # [P, 2, 512] = 2 PSUM banks, double-buffered across
                    # (jc, it) so scores MMs overlap the previous ACTIVATE
                    sc = sc_ps.tile([P, 2, T5], f32, tag="sc",
                                    name=f"sc_{hp}_{jc}_{it}")
                    nc.tensor.matmul(
                        sc[:, 0, :],
                        kT_sb[0:DH, hp, jc * P:(jc + 1) * P],
                        qT_sb[0:DH, hp, it * T5:(it + 1) * T5],
                        start=True, stop=True,
                    )
                    nc.tensor.matmul(
                        sc[:, 1, :],
                        kT_sb[DH:P, hp, jc * P:(jc + 1) * P],
                        qT_sb[DH:P, hp, it * T5:(it + 1) * T5],
                        start=True, stop=True,
                    )
                    es = es_pool.tile([P, 2, T5], bf16, tag="es",
                                      name=f"es_{hp}_{jc}_{it}")
                    nc.scalar.activation(out=es, in_=sc, func=Exp)
                    nc.vector.tensor_mul(
                        out=pt[:, jc, :, it * T5:(it + 1) * T5],
                        in0=es,
                        in1=s_t[:, :, off + it * T5:off + (it + 1) * T5])
                for fn in slots[jc]:
                    fn()

        prefetch_skew(0, eng=nc.scalar)
        # whole-W load: contiguous destination -> 128 large descriptors
        # (a column-sliced dst costs ~16ns/descriptor of HWDGE issue time)
        nc.scalar.dma_start(out=w_sb, in_=w_r)
        nc.scalar.dma_start(out=wp_sb, in_=wp_d.ap().rearrange("(kc p) d -> p kc d", p=P))
        bp_ap = bp_d.ap()
        bp_bcast = bass.AP(tensor=bp_ap.tensor, offset=bp_ap.offset,
                           ap=[[0, P], *bp_ap.ap])
        nc.gpsimd.dma_start(out=bp_sb, in_=bp_bcast)
        # dummy exp: pulls the ACT table load off the critical path
        dummy = rec_pool.tile([1, 1], f32, tag="rec", name="act_warm")
        nc.scalar.activation(out=dummy, in_=ident[0:1, 0:1], func=Exp)

        for which in range(2):
            for it in range(NT):
                qk_group(0, which, it)()

        pts = [None] * HP
        vg = [v_group(nj, et) for nj in range(NJ) for et in range(2)]
        for g in vg[:8]:
            g()
        vrest = vg[8:]

        fl = lambda: flush_tp(lag=1)
        for hp in range(HP):
            if hp + 1 < HP:
                prefetch_skew(hp + 1)
            slots = [[] for _ in range(NJ)]
            if hp + 1 < HP:
                qks = [qk_group(hp + 1, w, it)
                       for w in range(2) for it in range(NT)]
                slots[1].append(qks[0])
                slots[3].append(qks[1])
                slots[5].append(qks[2])
                slots[7].append(qks[3])
            if hp > 0:
                for k in range(NJ):
                    slots[k].insert(0, pv_group(hp - 1, k))
                    slots[k].append(fl)
            else:
                for k in range(NJ):
                    slots[k].insert(0, vrest[k])
            pts[hp] = pt_pool.tile([P, NJ, 2, N], bf16, tag="pt",
                                   name=f"pt_{hp}")
            scores_phase(hp, pts[hp], slots)

        # tail: PV(last pair) -> transpose -> proj, pipelined per i-block
        def flush_until(hp, ib):
            while pend:
                h0, i0, o_n = pend.pop(0)
                tp = mm_ps.tile([P, P], bf16, tag="mm", name=f"tp_{h0}_{i0}")
                nc.tensor.transpose(
                    tp, o_n[:, :, :].rearrange("p t d -> p (t d)"), ident)
                nc.vector.tensor_copy(
                    out=oT_sb[:, h0, i0 * P:(i0 + 1) * P], in_=tp)
                if h0 == hp and i0 == ib:
                    break

        for ib in range(NJ):
            pv_group(HP - 1, ib)()
            if ib >= 1:
                flush_until(HP - 1, ib - 1)
                proj_group(ib - 1)()
        flush_until(HP - 1, NJ - 1)
        proj_group(NJ - 1)()

    nc.finalize()
    return nc


def _get_nc():
    if "nc" not in _BUILT:
        _BUILT["nc"] = _build_nc()
    return _BUILT["nc"]


def _prep_inputs(x, W_qkv, W_proj, b_proj, bias_table, rel_index):
    bf = ml_dtypes.bfloat16
    x = np.asarray(x, dtype=np.float32)
    W_qkv = np.asarray(W_qkv, dtype=np.float32)
    W_proj = np.asarray(W_proj, dtype=np.float32)
    b_proj = np.asarray(b_proj, dtype=np.float32)
    bias_table = np.asarray(bias_table, dtype=np.float32)

    xT = np.ascontiguousarray(x.transpose(0, 2, 1)).astype(bf)       # [B, C, N]
    wq = W_qkv.copy()
    wq[:, :C] *= DH ** -0.5          # fold the attention scale into W_q
    wq = wq.astype(bf)
    wp = W_proj.astype(bf)

    # skewed multiplier table: S[h, p, u] = exp(t_h[p - u + 1919]);
    # E[h, j=128*jc+p, i] == S[h, p, i + 896 - 128*jc]
    t_exp = np.exp(bias_table)                      # [2047, H]
    idx = np.arange(P)[:, None] - np.arange(SW)[None, :] + 1919  # [P, SW]
    sk = t_exp[idx, :]                              # [P, SW, H]
    sk = np.ascontiguousarray(sk.transpose(2, 0, 1))  # [H, P, SW]
    sk = sk.reshape(HP, 2, P, SW).astype(bf)

    shared = {"wqkv": wq, "wproj": wp, "bproj": b_proj, "skew": sk}
    in_maps = []
    for b in range(B):
        m = dict(shared)
        m["xT"] = np.ascontiguousarray(xT[b])
        in_maps.append(m)
    return in_maps


def run(x, W_qkv, W_proj, b_proj, bias_table, rel_index, trace=False):
    """Returns (output [B, N, C] f32, exec_time_ns or None)."""
    from concourse.bass_utils import run_bass_kernel_spmd

    nc = _get_nc()
    in_maps = _prep_inputs(x, W_qkv, W_proj, b_proj, bias_table, rel_index)
    res = run_bass_kernel_spmd(nc, in_maps, core_ids=list(range(B)), trace=trace)
    out = np.stack([r["out"] for r in res.results]).astype(np.float32)
    return out, res.exec_time_ns


def kernel(x, W_qkv, W_proj, b_proj, bias_table, rel_index):
    out, _ = run(x, W_qkv, W_proj, b_proj, bias_table, rel_index, trace=False)
    return out


# revision 18
# speedup vs baseline: 1.2255x; 1.0389x over previous
"""Trainium2 Bass kernel for ViT-style multi-head attention with relative
position bias.

Problem (per full input):
  x        [8, 1024, 768] f32
  W_qkv    [768, 2304]    f32
  W_proj   [768, 768]     f32
  b_proj   [768]          f32
  bias_table [2047, 12]   f32
  rel_index  [1024, 1024] int32

Sharding: pure data parallel - one batch element per NeuronCore (B=8 over 8
cores), weights replicated. No collectives.

v2 design (vs v1 baseline at ~314us):
  - The rel-pos multiplier E[h, j, i] = exp(t_h[i - j + 1023]) is Toeplitz,
    so instead of streaming the 25MB expanded table from HBM, the host
    builds a SKEWED table S[h, p, u] = exp(t_h[u - p + 127]) of width 1920.
    Then E for score block (jc) is the plain slice S[h, :, off:off+1024]
    with off = 896 - 128*jc -- a uniform AP, zero expansion cost. Streamed
    per head-pair (983KB) instead of 25MB total.
  - PV is FLIPPED: stationary = pT chunk [j, i-block], moving = v [j, d+1]
    (with a ones column), giving out[i, d] per 128-i-block with the softmax
    denominator landing in a PSUM COLUMN -> per-partition reciprocal
    ([128,2] DVE op) + stride-0-broadcast tensor_mul normalize. This kills
    the v1 DRAM-bounce reciprocal dance (96 small DMAs).
  - The normalized o[i, c] tiles are PE-transposed ([128,128] via identity)
    into outT[c, i] for the projection, pipelined one slot behind PV so the
    PE never waits on the DVE chain.
  - exp runs as one ACTIVATE per (pair, jc) at FD=2048 from a 4-bank PSUM
    tile (amortizes the 352-cycle ACT overhead).
  - Tail: PV(last pair, ib) -> transpose -> proj(nj=ib) pipelined.

Emission is software-pipelined at head-pair granularity: scores(hp) with
filler slots running PV(hp-1), qk(hp+1), transposes. Matmul groups must NOT
be emitted between the row-tiled K=64 score matmuls of a pair (HW hazard);
whole groups at jc boundaries are safe.
"""

import numpy as np
import ml_dtypes

B = 8
N = 1024
C = 768
H = 12
DH = 64
P = 128
KC = C // P          # 6 contraction chunks of 128 over C
NJ = N // P          # 8 chunks of 128 over the j (key) axis
NT = N // 512        # 2 tiles of 512 over the i (query) axis
HP = H // 2          # 6 head pairs
T5 = 512
SW = 1920            # skewed-table width

_BUILT = {}


def _build_nc():
    from contextlib import ExitStack
    import concourse.bass as bass
    import concourse.mybir as mybir
    import concourse.tile as tile
    from concourse import bacc
    from concourse import masks

    bf16 = mybir.dt.bfloat16
    f32 = mybir.dt.float32
    Exp = mybir.ActivationFunctionType.Exp

    nc = bacc.Bacc("TRN2", target_bir_lowering=False, debug=False)

    xT_d = nc.dram_tensor("xT", [C, N], bf16, kind="ExternalInput")
    w_d = nc.dram_tensor("wqk", [C, 2 * C], bf16, kind="ExternalInput")
    wv_d = nc.dram_tensor("wv", [C, C], bf16, kind="ExternalInput")
    wp_d = nc.dram_tensor("wproj", [C, C], bf16, kind="ExternalInput")
    bp_d = nc.dram_tensor("bproj", [C], f32, kind="ExternalInput")
    sk_d = nc.dram_tensor("skew", [HP, 2, P, SW], bf16, kind="ExternalInput")
    out_d = nc.dram_tensor("out", [N, C], f32, kind="ExternalOutput")

    with ExitStack() as ctx:
        tc = ctx.enter_context(tile.TileContext(nc))

        singles = ctx.enter_context(tc.tile_pool(name="singles", bufs=1))
        pt_pool = ctx.enter_context(tc.tile_pool(name="pt_pool", bufs=2))
        s_pool = ctx.enter_context(tc.tile_pool(name="s_pool", bufs=2))
        es_pool = ctx.enter_context(tc.tile_pool(name="es_pool", bufs=3))
        on_pool = ctx.enter_context(tc.tile_pool(name="on_pool", bufs=3))
        rec_pool = ctx.enter_context(tc.tile_pool(name="rec_pool", bufs=3))
        ost_pool = ctx.enter_context(tc.tile_pool(name="ost_pool", bufs=2))
        sc_ps = ctx.enter_context(tc.tile_pool(name="sc_ps", bufs=2, space="PSUM"))
        mm_ps = ctx.enter_context(tc.tile_pool(name="mm_ps", bufs=2, space="PSUM"))
        pv_ps = ctx.enter_context(tc.tile_pool(name="pv_ps", bufs=2, space="PSUM"))

        # ---- resident SBUF tensors (loads chunked so the PE starts early) --
        xT_sb = singles.tile([P, KC, N], bf16)
        xT_r = xT_d.ap().rearrange("(kc p) n -> p kc n", p=P)
        w_sb = singles.tile([P, KC, 2 * C], bf16)
        w_r = w_d.ap().rearrange("(kc p) d -> p kc d", p=P)
        wv_sb = singles.tile([P, KC, C], bf16)
        wv_r = wv_d.ap().rearrange("(kc p) d -> p kc d", p=P)
        # input DMAs split across the two HWDGE rings (sync + scalar) with
        # contiguous destinations (a strided dst costs ~16ns/descriptor of
        # issue time); wqk in two halves so qk(0) starts on the first half
        nc.sync.dma_start(out=xT_sb, in_=xT_r)
        nc.scalar.dma_start(out=w_sb[:, 0:3], in_=w_r[:, 0:3])
        nc.scalar.dma_start(out=w_sb[:, 3:], in_=w_r[:, 3:])
        wp_sb = singles.tile([P, KC, C], bf16)
        bp_sb = singles.tile([P, C], f32)

        ident = singles.tile([P, P], bf16)
        masks.make_identity(nc, ident[:, :])

        qT_sb = singles.tile([P, HP, N], bf16)   # chunk hp = heads (2hp, 2hp+1)
        kT_sb = singles.tile([P, HP, N], bf16)
        v_sb = singles.tile([P, NJ, H, DH + 1], bf16)  # col DH = ones
        nc.vector.memset(v_sb[:, :, :, DH:DH + 1], 1.0)
        oT_sb = singles.tile([P, KC, N], bf16)   # kc chunk == pair hp

        # skew-table tiles, prefetched one pair ahead; pair 0 rides the
        # scalar HWDGE ring so it lands before the first TT multiply
        s_tiles = [None] * HP
        def prefetch_skew(hp, eng=None):
            s_tiles[hp] = s_pool.tile([P, 2, SW], bf16, tag="sk",
                                      name=f"sk_{hp}")
            (eng or nc.sync).dma_start(
                out=s_tiles[hp],
                in_=sk_d.ap()[hp].rearrange("t p u -> p t u"))

        # ---- matmul group emitters (closures; emitted in interleaved order) --

        def v_group(nj, et):
            e0 = et * 384
            def emit():
                ps = mm_ps.tile([P, 384], f32, tag="mm", name=f"ps_v_{nj}_{et}")
                for kc in range(KC):
                    nc.tensor.matmul(
                        ps,
                        xT_sb[:, kc, nj * P:(nj + 1) * P],
                        wv_sb[:, kc, e0:e0 + 384],
                        start=(kc == 0), stop=(kc == KC - 1),
                    )
                h0 = e0 // DH
                nc.vector.tensor_copy(
                    out=v_sb[:, nj, h0:h0 + 6, 0:DH],
                    in_=ps.rearrange("p (h d) -> p h d", h=6),
                )
            return emit

        def qk_group(hp, which, it):
            col0 = hp * P if which == 0 else C + hp * P
            def emit():
                d = qT_sb if which == 0 else kT_sb
                ps = mm_ps.tile([P, T5], f32, tag="mm",
                                name=f"ps_qk_{hp}_{which}_{it}")
                for kc in range(KC):
                    nc.tensor.matmul(
                        ps,
                        w_sb[:, kc, col0:col0 + P],
                        xT_sb[:, kc, it * T5:(it + 1) * T5],
                        start=(kc == 0), stop=(kc == KC - 1),
                    )
                nc.vector.tensor_copy(
                    out=d[:, hp, it * T5:(it + 1) * T5], in_=ps)
            return emit

        # pending (hp, ib, o_n) tiles awaiting PE transpose into oT_sb
        pend = []

        def pv_group(hp, ib):
            """Flipped PV for both heads of pair hp, query block ib."""
            def emit():
                pt = pts[hp]
                pv = pv_ps.tile([P, 2, DH + 1], f32, tag="pv",
                                name=f"pv_{hp}_{ib}")
                for t in range(2):
                    h = 2 * hp + t
                    for jc in range(NJ):
                        nc.tensor.matmul(
                            pv[:, t, :],
                            pt[:, jc, t, ib * P:(ib + 1) * P],
                            v_sb[:, jc, h, :],
                            start=(jc == 0), stop=(jc == NJ - 1),
                        )
                rec = rec_pool.tile([P, 2, 1], f32, tag="rec",
                                    name=f"rec_{hp}_{ib}")
                nc.vector.reciprocal(rec, pv[:, :, DH:DH + 1])
                o_n = on_pool.tile([P, 2, DH], bf16, tag="on",
                                   name=f"on_{hp}_{ib}")
                nc.vector.tensor_mul(
                    out=o_n,
                    in0=pv[:, :, 0:DH],
                    in1=rec[:, :, :].broadcast_to([P, 2, DH]),
                )
                pend.append((hp, ib, o_n))
            return emit

        def flush_tp(lag=1):
            """Transpose+evict the oldest pending o_n tile (keeping `lag`
            entries so the PE never waits on the DVE normalize chain)."""
            while len(pend) > lag:
                hp, ib, o_n = pend.pop(0)
                tp = mm_ps.tile([P, P], bf16, tag="mm", name=f"tp_{hp}_{ib}")
                nc.tensor.transpose(
                    tp, o_n[:, :, :].rearrange("p t d -> p (t d)"), ident)
                nc.vector.tensor_copy(
                    out=oT_sb[:, hp, ib * P:(ib + 1) * P], in_=tp)

        def proj_group(nj):
            # proj PSUM comes from sc_ps (dead after the last scores phase)
            # so the et groups double-buffer instead of waiting on bias-adds
            def emit():
                osb = ost_pool.tile([P, C], f32, tag="osb",
                                    name=f"osb_{nj}")
                for et in range(2):
                    pp = sc_ps.tile([P, 2, T5], f32, tag="sc",
                                    name=f"pp_{nj}_{et}")[:, 0, 0:384]
                    for kc in range(KC):
                        nc.tensor.matmul(
                            pp,
                            oT_sb[:, kc, nj * P:(nj + 1) * P],
                            wp_sb[:, kc, et * 384:(et + 1) * 384],
                            start=(kc == 0), stop=(kc == KC - 1),
                        )
                    nc.vector.tensor_add(
                        out=osb[:, et * 384:(et + 1) * 384],
                        in0=pp,
                        in1=bp_sb[:, et * 384:(et + 1) * 384],
                    )
                nc.sync.dma_start(
                    out=out_d.ap()[nj * P:(nj + 1) * P, :], in_=osb)
            return emit

        # ---- emission: software-pipelined at pair granularity ----
        # Per steady-state pair hp: scores(hp) jc-groups with slots running
        # PV(hp-1), qk(hp+1), and deferred transposes. NOTE: nothing may be
        # emitted BETWEEN the row-tiled K=64 score matmuls of a jc group.

        def scores_phase(hp, pt, slots):
            s_t = s_tiles[hp]
            for jc in range(NJ):
                off = 896 - 128 * jc
                for it in range(NT):
                    # [P, 2, 512] = 2 PSUM banks, double-buffered across
                    # (jc, it) so scores MMs overlap the previous ACTIVATE
                    sc = sc_ps.tile([P, 2, T5], f32, tag="sc",
                                    name=f"sc_{hp}_{jc}_{it}")
                    nc.tensor.matmul(
                        sc[:, 0, :],
                        kT_sb[0:DH, hp, jc * P:(jc + 1) * P],
                        qT_sb[0:DH, hp, it * T5:(it + 1) * T5],
                        start=True, stop=True,
                    )
                    nc.tensor.matmul(
                        sc[:, 1, :],
                        kT_sb[DH:P, hp, jc * P:(jc + 1) * P],
                        qT_sb[DH:P, hp, it * T5:(it + 1) * T5],
                        start=True, stop=True,
                    )
                    es = es_pool.tile([P, 2, T5], bf16, tag="es",
                                      name=f"es_{hp}_{jc}_{it}")
                    nc.scalar.activation(out=es, in_=sc, func=Exp)
                    nc.vector.tensor_mul(
                        out=pt[:, jc, :, it * T5:(it + 1) * T5],
                        in0=es,
                        in1=s_t[:, :, off + it * T5:off + (it + 1) * T5])
                for fn in slots[jc]:
                    fn()

        prefetch_skew(0)
        nc.scalar.dma_start(out=wv_sb, in_=wv_r)
        nc.scalar.dma_start(out=wp_sb, in_=wp_d.ap().rearrange("(kc p) d -> p kc d", p=P))
        bp_ap = bp_d.ap()
        bp_bcast = bass.AP(tensor=bp_ap.tensor, offset=bp_ap.offset,
                           ap=[[0, P], *bp_ap.ap])
        nc.gpsimd.dma_start(out=bp_sb, in_=bp_bcast)
        # dummy exp: pulls the ACT table load off the critical path
        dummy = rec_pool.tile([1, 1], f32, tag="rec", name="act_warm")
        nc.scalar.activation(out=dummy, in_=ident[0:1, 0:1], func=Exp)

        for which in range(2):
            for it in range(NT):
                qk_group(0, which, it)()

        pts = [None] * HP
        vg = [v_group(nj, et) for nj in range(NJ) for et in range(2)]
        for g in vg[:8]:
            g()
        vrest = vg[8:]

        fl = lambda: flush_tp(lag=1)
        for hp in range(HP):
            if hp + 1 < HP:
                prefetch_skew(hp + 1)
            slots = [[] for _ in range(NJ)]
            if hp + 1 < HP:
                qks = [qk_group(hp + 1, w, it)
                       for w in range(2) for it in range(NT)]
                slots[1].append(qks[0])
                slots[3].append(qks[1])
                slots[5].append(qks[2])
                slots[7].append(qks[3])
            if hp > 0:
                for k in range(NJ):
                    slots[k].insert(0, pv_group(hp - 1, k))
                    slots[k].append(fl)
            else:
                for k in range(NJ):
                    slots[k].insert(0, vrest[k])
            pts[hp] = pt_pool.tile([P, NJ, 2, N], bf16, tag="pt",
                                   name=f"pt_{hp}")
            scores_phase(hp, pts[hp], slots)

        # tail: PV(last pair) -> transpose -> proj, pipelined per i-block
        def flush_until(hp, ib):
            while pend:
                h0, i0, o_n = pend.pop(0)
                tp = mm_ps.tile([P, P], bf16, tag="mm", name=f"tp_{h0}_{i0}")
                nc.tensor.transpose(
                    tp, o_n[:, :, :].rearrange("p t d -> p (t d)"), ident)
                nc.vector.tensor_copy(
                    out=oT_sb[:, h0, i0 * P:(i0 + 1) * P], in_=tp)
                if h0 == hp and i0 == ib:
                    break

        for ib in range(NJ):
            pv_group(HP - 1, ib)()
            if ib >= 1:
                flush_until(HP - 1, ib - 1)
                proj_group(ib - 1)()
        flush_until(HP - 1, NJ - 1)
        proj_group(NJ - 1)()

    nc.finalize()
    return nc


def _get_nc():
    if "nc" not in _BUILT:
        _BUILT["nc"] = _build_nc()
    return _BUILT["nc"]


def _prep_inputs(x, W_qkv, W_proj, b_proj, bias_table, rel_index):
    bf = ml_dtypes.bfloat16
    x = np.asarray(x, dtype=np.float32)
    W_qkv = np.asarray(W_qkv, dtype=np.float32)
    W_proj = np.asarray(W_proj, dtype=np.float32)
    b_proj = np.asarray(b_proj, dtype=np.float32)
    bias_table = np.asarray(bias_table, dtype=np.float32)

    xT = np.ascontiguousarray(x.transpose(0, 2, 1)).astype(bf)       # [B, C, N]
    wq = W_qkv.copy()
    wq[:, :C] *= DH ** -0.5          # fold the attention scale into W_q
    wqk = np.ascontiguousarray(wq[:, :2 * C]).astype(bf)
    wv = np.ascontiguousarray(wq[:, 2 * C:]).astype(bf)
    wp = W_proj.astype(bf)

    # skewed multiplier table: S[h, p, u] = exp(t_h[p - u + 1919]);
    # E[h, j=128*jc+p, i] == S[h, p, i + 896 - 128*jc]
    t_exp = np.exp(bias_table)                      # [2047, H]
    idx = np.arange(P)[:, None] - np.arange(SW)[None, :] + 1919  # [P, SW]
    sk = t_exp[idx, :]                              # [P, SW, H]
    sk = np.ascontiguousarray(sk.transpose(2, 0, 1))  # [H, P, SW]
    sk = sk.reshape(HP, 2, P, SW).astype(bf)

    shared = {"wqk": wqk, "wv": wv, "wproj": wp, "bproj": b_proj, "skew": sk}
    in_maps = []
    for b in range(B):
        m = dict(shared)
        m["xT"] = np.ascontiguousarray(xT[b])
        in_maps.append(m)
    return in_maps


def run(x, W_qkv, W_proj, b_proj, bias_table, rel_index, trace=False):
    """Returns (output [B, N, C] f32, exec_time_ns or None)."""
    from concourse.bass_utils import run_bass_kernel_spmd

    nc = _get_nc()
    in_maps = _prep_inputs(x, W_qkv, W_proj, b_proj, bias_table, rel_index)
    res = run_bass_kernel_spmd(nc, in_maps, core_ids=list(range(B)), trace=trace)
    out = np.stack([r["out"] for r in res.results]).astype(np.float32)
    return out, res.exec_time_ns


def kernel(x, W_qkv, W_proj, b_proj, bias_table, rel_index):
    out, _ = run(x, W_qkv, W_proj, b_proj, bias_table, rel_index, trace=False)
    return out


# revision 19
# speedup vs baseline: 1.2272x; 1.0014x over previous
"""Trainium2 Bass kernel for ViT-style multi-head attention with relative
position bias.

Problem (per full input):
  x        [8, 1024, 768] f32
  W_qkv    [768, 2304]    f32
  W_proj   [768, 768]     f32
  b_proj   [768]          f32
  bias_table [2047, 12]   f32
  rel_index  [1024, 1024] int32

Sharding: pure data parallel - one batch element per NeuronCore (B=8 over 8
cores), weights replicated. No collectives.

v2 design (vs v1 baseline at ~314us):
  - The rel-pos multiplier E[h, j, i] = exp(t_h[i - j + 1023]) is Toeplitz,
    so instead of streaming the 25MB expanded table from HBM, the host
    builds a SKEWED table S[h, p, u] = exp(t_h[u - p + 127]) of width 1920.
    Then E for score block (jc) is the plain slice S[h, :, off:off+1024]
    with off = 896 - 128*jc -- a uniform AP, zero expansion cost. Streamed
    per head-pair (983KB) instead of 25MB total.
  - PV is FLIPPED: stationary = pT chunk [j, i-block], moving = v [j, d+1]
    (with a ones column), giving out[i, d] per 128-i-block with the softmax
    denominator landing in a PSUM COLUMN -> per-partition reciprocal
    ([128,2] DVE op) + stride-0-broadcast tensor_mul normalize. This kills
    the v1 DRAM-bounce reciprocal dance (96 small DMAs).
  - The normalized o[i, c] tiles are PE-transposed ([128,128] via identity)
    into outT[c, i] for the projection, pipelined one slot behind PV so the
    PE never waits on the DVE chain.
  - exp runs as one ACTIVATE per (pair, jc) at FD=2048 from a 4-bank PSUM
    tile (amortizes the 352-cycle ACT overhead).
  - Tail: PV(last pair, ib) -> transpose -> proj(nj=ib) pipelined.

Emission is software-pipelined at head-pair granularity: scores(hp) with
filler slots running PV(hp-1), qk(hp+1), transposes. Matmul groups must NOT
be emitted between the row-tiled K=64 score matmuls of a pair (HW hazard);
whole groups at jc boundaries are safe.
"""

import numpy as np
import ml_dtypes

B = 8
N = 1024
C = 768
H = 12
DH = 64
P = 128
KC = C // P          # 6 contraction chunks of 128 over C
NJ = N // P          # 8 chunks of 128 over the j (key) axis
NT = N // 512        # 2 tiles of 512 over the i (query) axis
HP = H // 2          # 6 head pairs
T5 = 512
SW = 1920            # skewed-table width

_BUILT = {}


def _build_nc():
    from contextlib import ExitStack
    import concourse.bass as bass
    import concourse.mybir as mybir
    import concourse.tile as tile
    from concourse import bacc
    from concourse import masks

    bf16 = mybir.dt.bfloat16
    f32 = mybir.dt.float32
    Exp = mybir.ActivationFunctionType.Exp

    nc = bacc.Bacc("TRN2", target_bir_lowering=False, debug=False)

    xT_d = nc.dram_tensor("xT", [C, N], bf16, kind="ExternalInput")
    w_d = nc.dram_tensor("wqk", [C, 2 * C], bf16, kind="ExternalInput")
    wv_d = nc.dram_tensor("wv", [C, C], bf16, kind="ExternalInput")
    wp_d = nc.dram_tensor("wproj", [C, C], bf16, kind="ExternalInput")
    bp_d = nc.dram_tensor("bproj", [C], f32, kind="ExternalInput")
    sk_d = nc.dram_tensor("skew", [HP, 2, P, SW], bf16, kind="ExternalInput")
    out_d = nc.dram_tensor("out", [N, C], f32, kind="ExternalOutput")

    with ExitStack() as ctx:
        tc = ctx.enter_context(tile.TileContext(nc))

        singles = ctx.enter_context(tc.tile_pool(name="singles", bufs=1))
        pt_pool = ctx.enter_context(tc.tile_pool(name="pt_pool", bufs=2))
        s_pool = ctx.enter_context(tc.tile_pool(name="s_pool", bufs=2))
        es_pool = ctx.enter_context(tc.tile_pool(name="es_pool", bufs=3))
        on_pool = ctx.enter_context(tc.tile_pool(name="on_pool", bufs=3))
        rec_pool = ctx.enter_context(tc.tile_pool(name="rec_pool", bufs=3))
        ost_pool = ctx.enter_context(tc.tile_pool(name="ost_pool", bufs=2))
        sc_ps = ctx.enter_context(tc.tile_pool(name="sc_ps", bufs=2, space="PSUM"))
        mm_ps = ctx.enter_context(tc.tile_pool(name="mm_ps", bufs=2, space="PSUM"))
        pv_ps = ctx.enter_context(tc.tile_pool(name="pv_ps", bufs=2, space="PSUM"))

        # ---- resident SBUF tensors (loads chunked so the PE starts early) --
        xT_sb = singles.tile([P, KC, N], bf16)
        xT_r = xT_d.ap().rearrange("(kc p) n -> p kc n", p=P)
        w_sb = singles.tile([P, KC, 2 * C], bf16)
        w_r = w_d.ap().rearrange("(kc p) d -> p kc d", p=P)
        wv_sb = singles.tile([P, KC, C], bf16)
        wv_r = wv_d.ap().rearrange("(kc p) d -> p kc d", p=P)
        # input DMAs split across the two HWDGE rings (sync + scalar) with
        # contiguous destinations (a strided dst costs ~16ns/descriptor of
        # issue time); wqk in two halves so qk(0) starts on the first half
        nc.sync.dma_start(out=xT_sb, in_=xT_r)
        nc.scalar.dma_start(out=w_sb[:, 0:3], in_=w_r[:, 0:3])
        nc.scalar.dma_start(out=w_sb[:, 3:], in_=w_r[:, 3:])
        wp_sb = singles.tile([P, KC, C], bf16)
        bp_sb = singles.tile([P, C], f32)

        ident = singles.tile([P, P], bf16)
        masks.make_identity(nc, ident[:, :])

        qT_sb = singles.tile([P, HP, N], bf16)   # chunk hp = heads (2hp, 2hp+1)
        kT_sb = singles.tile([P, HP, N], bf16)
        v_sb = singles.tile([P, NJ, H, DH + 1], bf16)  # col DH = ones
        nc.vector.memset(v_sb[:, :, :, DH:DH + 1], 1.0)
        oT_sb = singles.tile([P, KC, N], bf16)   # kc chunk == pair hp

        # skew-table tiles, prefetched one pair ahead; pair 0 rides the
        # scalar HWDGE ring so it lands before the first TT multiply
        s_tiles = [None] * HP
        def prefetch_skew(hp, eng=None):
            s_tiles[hp] = s_pool.tile([P, 2, SW], bf16, tag="sk",
                                      name=f"sk_{hp}")
            (eng or nc.sync).dma_start(
                out=s_tiles[hp],
                in_=sk_d.ap()[hp].rearrange("t p u -> p t u"))

        # ---- matmul group emitters (closures; emitted in interleaved order) --

        def v_group(nj, et):
            e0 = et * 384
            def emit():
                ps = mm_ps.tile([P, 384], f32, tag="mm", name=f"ps_v_{nj}_{et}")
                for kc in range(KC):
                    nc.tensor.matmul(
                        ps,
                        xT_sb[:, kc, nj * P:(nj + 1) * P],
                        wv_sb[:, kc, e0:e0 + 384],
                        start=(kc == 0), stop=(kc == KC - 1),
                    )
                h0 = e0 // DH
                nc.vector.tensor_copy(
                    out=v_sb[:, nj, h0:h0 + 6, 0:DH],
                    in_=ps.rearrange("p (h d) -> p h d", h=6),
                )
            return emit

        def qk_group(hp, which, it):
            col0 = hp * P if which == 0 else C + hp * P
            def emit():
                d = qT_sb if which == 0 else kT_sb
                ps = mm_ps.tile([P, T5], f32, tag="mm",
                                name=f"ps_qk_{hp}_{which}_{it}")
                for kc in range(KC):
                    nc.tensor.matmul(
                        ps,
                        w_sb[:, kc, col0:col0 + P],
                        xT_sb[:, kc, it * T5:(it + 1) * T5],
                        start=(kc == 0), stop=(kc == KC - 1),
                    )
                nc.vector.tensor_copy(
                    out=d[:, hp, it * T5:(it + 1) * T5], in_=ps)
            return emit

        # pending (hp, ib, o_n) tiles awaiting PE transpose into oT_sb
        pend = []

        def pv_group(hp, ib):
            """Flipped PV for both heads of pair hp, query block ib."""
            def emit():
                pt = pts[hp]
                pv = pv_ps.tile([P, 2, DH + 1], f32, tag="pv",
                                name=f"pv_{hp}_{ib}")
                for t in range(2):
                    h = 2 * hp + t
                    for jc in range(NJ):
                        nc.tensor.matmul(
                            pv[:, t, :],
                            pt[:, jc, t, ib * P:(ib + 1) * P],
                            v_sb[:, jc, h, :],
                            start=(jc == 0), stop=(jc == NJ - 1),
                        )
                rec = rec_pool.tile([P, 2, 1], f32, tag="rec",
                                    name=f"rec_{hp}_{ib}")
                nc.vector.reciprocal(rec, pv[:, :, DH:DH + 1])
                o_n = on_pool.tile([P, 2, DH], bf16, tag="on",
                                   name=f"on_{hp}_{ib}")
                nc.vector.tensor_mul(
                    out=o_n,
                    in0=pv[:, :, 0:DH],
                    in1=rec[:, :, :].broadcast_to([P, 2, DH]),
                )
                pend.append((hp, ib, o_n))
            return emit

        def flush_tp(lag=1):
            """Transpose+evict the oldest pending o_n tile (keeping `lag`
            entries so the PE never waits on the DVE normalize chain)."""
            while len(pend) > lag:
                hp, ib, o_n = pend.pop(0)
                tp = mm_ps.tile([P, P], bf16, tag="mm", name=f"tp_{hp}_{ib}")
                nc.tensor.transpose(
                    tp, o_n[:, :, :].rearrange("p t d -> p (t d)"), ident)
                nc.vector.tensor_copy(
                    out=oT_sb[:, hp, ib * P:(ib + 1) * P], in_=tp)

        def proj_group(nj):
            # proj PSUM comes from sc_ps (dead after the last scores phase)
            # so the et groups double-buffer instead of waiting on bias-adds
            def emit():
                osb = ost_pool.tile([P, C], f32, tag="osb",
                                    name=f"osb_{nj}")
                for et in range(2):
                    pp = sc_ps.tile([P, 2, T5], f32, tag="sc",
                                    name=f"pp_{nj}_{et}")[:, 0, 0:384]
                    for kc in range(KC):
                        nc.tensor.matmul(
                            pp,
                            oT_sb[:, kc, nj * P:(nj + 1) * P],
                            wp_sb[:, kc, et * 384:(et + 1) * 384],
                            start=(kc == 0), stop=(kc == KC - 1),
                        )
                    nc.vector.tensor_add(
                        out=osb[:, et * 384:(et + 1) * 384],
                        in0=pp,
                        in1=bp_sb[:, et * 384:(et + 1) * 384],
                    )
                nc.sync.dma_start(
                    out=out_d.ap()[nj * P:(nj + 1) * P, :], in_=osb)
            return emit

        # ---- emission: software-pipelined at pair granularity ----
        # Per steady-state pair hp: scores(hp) jc-groups with slots running
        # PV(hp-1), qk(hp+1), and deferred transposes. NOTE: nothing may be
        # emitted BETWEEN the row-tiled K=64 score matmuls of a jc group.

        def scores_phase(hp, pt, slots):
            s_t = s_tiles[hp]
            for jc in range(NJ):
                off = 896 - 128 * jc
                for it in range(NT):
                    # [P, 2, 512] = 2 PSUM banks, double-buffered across
                    # (jc, it) so scores MMs overlap the previous ACTIVATE
                    sc = sc_ps.tile([P, 2, T5], f32, tag="sc",
                                    name=f"sc_{hp}_{jc}_{it}")
                    nc.tensor.matmul(
                        sc[:, 0, :],
                        kT_sb[0:DH, hp, jc * P:(jc + 1) * P],
                        qT_sb[0:DH, hp, it * T5:(it + 1) * T5],
                        start=True, stop=True,
                    )
                    nc.tensor.matmul(
                        sc[:, 1, :],
                        kT_sb[DH:P, hp, jc * P:(jc + 1) * P],
                        qT_sb[DH:P, hp, it * T5:(it + 1) * T5],
                        start=True, stop=True,
                    )
                    es = es_pool.tile([P, 2, T5], bf16, tag="es",
                                      name=f"es_{hp}_{jc}_{it}")
                    nc.scalar.activation(out=es, in_=sc, func=Exp)
                    nc.vector.tensor_mul(
                        out=pt[:, jc, :, it * T5:(it + 1) * T5],
                        in0=es,
                        in1=s_t[:, :, off + it * T5:off + (it + 1) * T5])
                for fn in slots[jc]:
                    fn()

        prefetch_skew(0)
        nc.scalar.dma_start(out=wv_sb, in_=wv_r)
        nc.scalar.dma_start(out=wp_sb, in_=wp_d.ap().rearrange("(kc p) d -> p kc d", p=P))
        bp_ap = bp_d.ap()
        bp_bcast = bass.AP(tensor=bp_ap.tensor, offset=bp_ap.offset,
                           ap=[[0, P], *bp_ap.ap])
        nc.gpsimd.dma_start(out=bp_sb, in_=bp_bcast)
        # dummy exp: pulls the ACT table load off the critical path
        dummy = rec_pool.tile([1, 1], f32, tag="rec", name="act_warm")
        nc.scalar.activation(out=dummy, in_=ident[0:1, 0:1], func=Exp)
        # PE warm-up: ~4us of dummy matmuls on the identity tile while the
        # weight DMAs land, so HAM un-throttles before qk(0) issues
        warm_ps = pv_ps.tile([P, 2, DH + 1], f32, tag="pv", name="warm_ps")
        for _ in range(34):
            nc.tensor.matmul(warm_ps[:, 0, :], ident, ident[:, 0:DH + 1],
                             start=True, stop=True)

        for which in range(2):
            for it in range(NT):
                qk_group(0, which, it)()

        pts = [None] * HP
        vg = [v_group(nj, et) for nj in range(NJ) for et in range(2)]
        for g in vg[:8]:
            g()
        vrest = vg[8:]

        fl = lambda: flush_tp(lag=1)
        for hp in range(HP):
            if hp + 1 < HP:
                prefetch_skew(hp + 1)
            slots = [[] for _ in range(NJ)]
            if hp + 1 < HP:
                qks = [qk_group(hp + 1, w, it)
                       for w in range(2) for it in range(NT)]
                slots[1].append(qks[0])
                slots[3].append(qks[1])
                slots[5].append(qks[2])
                slots[7].append(qks[3])
            if hp > 0:
                for k in range(NJ):
                    slots[k].insert(0, pv_group(hp - 1, k))
                    slots[k].append(fl)
            else:
                for k in range(NJ):
                    slots[k].insert(0, vrest[k])
            pts[hp] = pt_pool.tile([P, NJ, 2, N], bf16, tag="pt",
                                   name=f"pt_{hp}")
            scores_phase(hp, pts[hp], slots)

        # tail: PV(last pair) -> transpose -> proj, pipelined per i-block
        def flush_until(hp, ib):
            while pend:
                h0, i0, o_n = pend.pop(0)
                tp = mm_ps.tile([P, P], bf16, tag="mm", name=f"tp_{h0}_{i0}")
                nc.tensor.transpose(
                    tp, o_n[:, :, :].rearrange("p t d -> p (t d)"), ident)
                nc.vector.tensor_copy(
                    out=oT_sb[:, h0, i0 * P:(i0 + 1) * P], in_=tp)
                if h0 == hp and i0 == ib:
                    break

        for ib in range(NJ):
            pv_group(HP - 1, ib)()
            if ib >= 1:
                flush_until(HP - 1, ib - 1)
                proj_group(ib - 1)()
        flush_until(HP - 1, NJ - 1)
        proj_group(NJ - 1)()

    nc.finalize()
    return nc


def _get_nc():
    if "nc" not in _BUILT:
        _BUILT["nc"] = _build_nc()
    return _BUILT["nc"]


def _prep_inputs(x, W_qkv, W_proj, b_proj, bias_table, rel_index):
    bf = ml_dtypes.bfloat16
    x = np.asarray(x, dtype=np.float32)
    W_qkv = np.asarray(W_qkv, dtype=np.float32)
    W_proj = np.asarray(W_proj, dtype=np.float32)
    b_proj = np.asarray(b_proj, dtype=np.float32)
    bias_table = np.asarray(bias_table, dtype=np.float32)

    xT = np.ascontiguousarray(x.transpose(0, 2, 1)).astype(bf)       # [B, C, N]
    wq = W_qkv.copy()
    wq[:, :C] *= DH ** -0.5          # fold the attention scale into W_q
    wqk = np.ascontiguousarray(wq[:, :2 * C]).astype(bf)
    wv = np.ascontiguousarray(wq[:, 2 * C:]).astype(bf)
    wp = W_proj.astype(bf)

    # skewed multiplier table: S[h, p, u] = exp(t_h[p - u + 1919]);
    # E[h, j=128*jc+p, i] == S[h, p, i + 896 - 128*jc]
    t_exp = np.exp(bias_table)                      # [2047, H]
    idx = np.arange(P)[:, None] - np.arange(SW)[None, :] + 1919  # [P, SW]
    sk = t_exp[idx, :]                              # [P, SW, H]
    sk = np.ascontiguousarray(sk.transpose(2, 0, 1))  # [H, P, SW]
    sk = sk.reshape(HP, 2, P, SW).astype(bf)

    shared = {"wqk": wqk, "wv": wv, "wproj": wp, "bproj": b_proj, "skew": sk}
    in_maps = []
    for b in range(B):
        m = dict(shared)
        m["xT"] = np.ascontiguousarray(xT[b])
        in_maps.append(m)
    return in_maps


def run(x, W_qkv, W_proj, b_proj, bias_table, rel_index, trace=False):
    """Returns (output [B, N, C] f32, exec_time_ns or None)."""
    from concourse.bass_utils import run_bass_kernel_spmd

    nc = _get_nc()
    in_maps = _prep_inputs(x, W_qkv, W_proj, b_proj, bias_table, rel_index)
    res = run_bass_kernel_spmd(nc, in_maps, core_ids=list(range(B)), trace=trace)
    out = np.stack([r["out"] for r in res.results]).astype(np.float32)
    return out, res.exec_time_ns


def kernel(x, W_qkv, W_proj, b_proj, bias_table, rel_index):
    out, _ = run(x, W_qkv, W_proj, b_proj, bias_table, rel_index, trace=False)
    return out
